# revision 39
# baseline (speedup 1.0000x reference)
"""Trainium2 Bass kernel for nn_AttentionHead (pre-softmax scores variant).

The module returns (q @ k^T * scale) @ v with NO softmax, so the product is
associative:  out = (scale*q) @ (k^T @ v)  with k^T @ v a tiny [64, 64]
matrix.  This removes the [T, T] score matrix entirely: the kernel streams
x once, computes k/v/q projections, a [64, 64] partial S = k^T v, a pairwise
AllGather+add across the two cores holding each batch, and one final
tall-skinny matmul.

Sharding: core c <- (batch b = c//2, sequence half h = c%2), 2048 tokens per
core.  Partial S matrices are exchanged within core pairs
[[0,1],[2,3],[4,5],[6,7]].

The wall-clock cost of a call is dominated by the axon tunnel (~60 MB/s
aggregate), so the host<->device byte count is the primary optimization
axis: x ships as fp16 (24 MB instead of 48), already transposed host-side
(the transpose fuses into the fp32->fp16 cast for free), weights/constants
are content-hashed and kept device-resident across calls, output buffers
are created on-device inside the jit, and the output returns as fp16
(2 MB).  fp16 keeps the end-to-end relative error ~5e-4.
"""

import hashlib
import sys

sys.path.insert(0, "/opt/trn_rl_repo")

import numpy as np

B, T, C, H = 4, 4096, 768, 64
N_CORES = 8
TPC = T // 2  # tokens per core (half a batch's sequence)
CI = C // 128  # 6 contraction chunks
NT = TPC // 512  # 4 moving-dim slices for projections
TI = TPC // 128  # 16 token tiles
SCALE = float(C) ** -0.5

_CACHE = {}


def _patch_tile_drain():
    """This walrus build rejects >1 sync wait on TPB_CTRL instructions
    (Drain/NoOp) and the butterfly barrier rides eq-waits on drains.
    Replace the TileContext exit sequence with single-wait nops + plain
    drain + sem-only barriers."""
    import bass_rust as _bass_rust
    import concourse.tile as tile
    from concourse.vector_clock import ScopedClock

    def _drain_and_barrier(self, tick_clock, wait_clock):
        nc = self.nc
        probe = nc.sync.nop(nofuse=True)
        wait_clock.add_sem_waits(
            probe.ins, ScopedClock({None: tick_clock.global_clock})
        )
        waits = list(probe.ins.sync_info.on_wait) if probe.ins.sync_info else []
        updates = list(probe.ins.sync_info.on_update) if probe.ins.sync_info else []
        probe.ins.sync_info = _bass_rust.SyncInfo(
            on_wait=waits[:1], on_update=updates
        )
        for i in range(1, len(waits)):
            extra = nc.sync.nop(nofuse=True)
            extra.ins.sync_info = _bass_rust.SyncInfo(
                on_wait=waits[i : i + 1], on_update=[]
            )
        nc.sync.drain()
        nc.all_engine_barrier(sem_only=True)
        popped = nc._tile_sem_poison_stack.pop()
        assert popped is self._sem_poison
        # Skip the exit-time semaphore clear + second barrier (several us of
        # counted exec span): the NEFF prologue re-initializes semaphore
        # state on every execution — repeat-call correctness was verified
        # with this exact exit sequence.

    tile.TileContext._drain_and_barrier = _drain_and_barrier


def _split_multi_waits(nc):
    """This walrus build allows only ONE sync-wait command per regular
    instruction.  Move extra waits onto dedicated same-engine NOPs placed
    immediately before the instruction (an engine blocks on its own stream,
    so this is semantically identical)."""
    import bass_rust
    import concourse.mybir as mybir

    cnt = 0
    for fn in nc.m.functions:
        for bb in fn.blocks:
            out = []
            for ins in bb.instructions:
                si = ins.sync_info
                if si is not None and si.on_wait and len(si.on_wait) > 1:
                    waits = list(si.on_wait)
                    for w in waits[:-1]:
                        nop = mybir.InstNoOp(name=f"I-waitsplit-{cnt}")
                        cnt += 1
                        nop.engine = ins.engine
                        nop.bass_nofuse = True
                        nop.sync_info = bass_rust.SyncInfo(
                            on_wait=[w], on_update=[]
                        )
                        out.append(nop)
                    ins.sync_info = bass_rust.SyncInfo(
                        on_wait=[waits[-1]], on_update=list(si.on_update or [])
                    )
                out.append(ins)
            bb.instructions = out
    return cnt


def _dedup_ldweights(nc):
    """Tile lowers every non-fp32 matmul into an LDWEIGHTS+MATMUL pair.
    When consecutive PE matmuls share the identical stationary operand the
    reload is redundant (the array already holds it) — delete those
    LDWEIGHTS, reattaching any sync waits to the next instruction."""
    import bass_rust

    def wkey(pap):
        return (str(pap.ap), pap.offset, str(pap.memref))

    removed = 0
    for fn in nc.m.functions:
        for bb in fn.blocks:
            out = []
            last_w = None
            pending_waits = []
            for ins in bb.instructions:
                nm = type(ins).__name__
                if nm == "InstLdweights":
                    k = wkey(ins.ins[0])
                    if last_w == k:
                        if ins.sync_info and ins.sync_info.on_wait:
                            pending_waits.extend(ins.sync_info.on_wait)
                        if ins.sync_info and ins.sync_info.on_update:
                            out.append(ins)
                            last_w = k
                            continue
                        removed += 1
                        continue
                    last_w = k
                elif nm == "InstMatmult":
                    if ins.is_transpose:
                        last_w = None  # transpose streams data through the array
                    else:
                        last_w = wkey(ins.ins[1])
                elif nm in ("InstCompareAndBranch", "InstUnconditionalBranch",
                            "InstCall", "InstDrain"):
                    last_w = None
                if pending_waits and ins.engine is not None:
                    w = list(pending_waits)
                    if ins.sync_info:
                        w = list(ins.sync_info.on_wait) + w
                        upd = list(ins.sync_info.on_update)
                    else:
                        upd = []
                    ins.sync_info = bass_rust.SyncInfo(on_wait=w, on_update=upd)
                    pending_waits = []
                out.append(ins)
            bb.instructions = out
    return removed


def _build_nc(no_collective=False, walrus_patches=True, pe_transpose=True):
    import concourse.bass as bass
    import concourse.mybir as mybir
    import concourse.tile as tile
    from bass_rust import add_dep_helper

    _patch_tile_drain()

    f32 = mybir.dt.float32
    f16 = mybir.dt.float16
    Identity = mybir.ActivationFunctionType.Identity

    nc = bass.Bass("TRN2", target_bir_lowering=False, debug=False,
                   num_devices=N_CORES)

    xt = nc.dram_tensor("xt", [128, CI, TPC], f16, kind="ExternalInput").ap()
    wkv = nc.dram_tensor("wkv", [128, CI, 128], f16, kind="ExternalInput").ap()
    wq = nc.dram_tensor("wq", [128, CI, H], f16, kind="ExternalInput").ap()
    bkv = nc.dram_tensor("bkv", [128, 1], f32, kind="ExternalInput").ap()
    bqp = nc.dram_tensor("bq", [H, 1], f32, kind="ExternalInput").ap()
    ident = nc.dram_tensor("ident", [128, 128], f16, kind="ExternalInput").ap()
    out = nc.dram_tensor("out", [128, 2, 8 * H], f16, kind="ExternalOutput").ap()
    cc_in = nc.dram_tensor("cc_in", [H, H], f16)
    cc_out = nc.dram_tensor("cc_out", [2, H, H], f16)
    cc_warm_in = nc.dram_tensor("cc_warm_in", [1, 1], f32)
    cc_warm_out = nc.dram_tensor("cc_warm_out", [2, 1], f32)
    RG = [[0, 1], [2, 3], [4, 5], [6, 7]]

    with tile.TileContext(nc) as tc:
        with (
            tc.tile_pool(name="const", bufs=1) as cpool,
            tc.tile_pool(name="data", bufs=1) as dpool,
            tc.tile_pool(name="work", bufs=2) as wpool,
            tc.tile_pool(name="psum", bufs=4, space="PSUM") as ppool,
        ):
            # DMA order tuned for the kv critical path: wkv heads the SP
            # queue so the first matmul is gated only on it + chunk 0;
            # chunk 0 lands in matmul-sized quarters so the PE starts
            # after 128 KB.  Even chunks ride SP, odd chunks + the
            # late-needed consts ride Act.  Each chunk gets its own tile
            # so matmuls gate on per-chunk DMA completion.
            # Warm-up collective: absorbs the CC-stream startup latency and
            # barrier-aligns the cores while x is still streaming, so the
            # real S exchange later pays only the transfer.  Contents are
            # irrelevant (uninitialized DRAM round-trip, never read).
            if not no_collective:
                nc.gpsimd.collective_compute(
                    "AllGather",
                    mybir.AluOpType.bypass,
                    replica_groups=RG,
                    ins=[cc_warm_in.ap()],
                    outs=[cc_warm_out.ap()],
                )

            wkv_sb = cpool.tile([128, CI, 128], f16)
            nc.sync.dma_start(out=wkv_sb[:], in_=wkv)
            bkv_sb = cpool.tile([128, 1], f32)
            nc.scalar.dma_start(out=bkv_sb[:], in_=bkv)
            bq_sb = cpool.tile([H, 1], f32)
            nc.scalar.dma_start(out=bq_sb[:], in_=bqp)
            wq_sb = cpool.tile([128, CI, H], f16)
            nc.scalar.dma_start(out=wq_sb[:], in_=wq)

            # Preload the Act engine's PWP table off the critical path (it
            # otherwise loads lazily right before the first bias ACTIVATE).
            act_dummy = cpool.tile([1, 1], f32)
            nc.scalar.activation(
                out=act_dummy[:], in_=bkv_sb[0:1, 0:1],
                func=Identity, bias=bkv_sb[0:1, 0:1],
            )

            # Queue balance: chunks 0-3 are split in half across BOTH HWDGE
            # queues (halving each chunk's arrival latency); the fast SWDGE
            # burst queue carries c4 + c5 whole.  The kv loop consumes
            # chunks in expected-arrival order.
            xt_sb = [dpool.tile([128, TPC], f16, name=f"xt{ci}") for ci in range(CI)]
            nc.gpsimd.dma_start(out=xt_sb[4][:], in_=xt[:, 4, :])
            nc.gpsimd.dma_start(out=xt_sb[5][:], in_=xt[:, 5, :])
            id_sb = cpool.tile([128, 128], f16)
            if pe_transpose:
                nc.scalar.dma_start(out=id_sb[:], in_=ident)
            for ci in range(4):
                nc.sync.dma_start(
                    out=xt_sb[ci][:, 0:1024], in_=xt[:, ci, 0:1024]
                )
                nc.scalar.dma_start(
                    out=xt_sb[ci][:, 1024:2048], in_=xt[:, ci, 1024:2048]
                )
            CI_ORDER = [0, 4, 1, 5, 2, 3]

            # kv^T = (Wk | Wv)^T x^T + bias (rows 0..63 k^T, 64..127 v^T).
            # kv first so the S collective launches before the q work.
            kvT = dpool.tile([128, TPC], f16)
            qT = dpool.tile([H, TPC], f16)
            psum_kv = [
                ppool.tile([128, 512], f32, tag="A", name=f"pkv{nt}")
                for nt in range(NT)
            ]
            psum_q = [
                ppool.tile([H, 512], f32, tag="B", name=f"pq{nt}")
                for nt in range(NT)
            ]
            for k, ci in enumerate(CI_ORDER):
                for nt in range(NT):
                    sl = slice(nt * 512, (nt + 1) * 512)
                    nc.tensor.matmul(
                        psum_kv[nt][:], wkv_sb[:, ci, :], xt_sb[ci][:, sl],
                        start=(k == 0), stop=(k == CI - 1),
                    )
            for nt in range(NT):
                sl = slice(nt * 512, (nt + 1) * 512)
                if nt % 2 == 0:
                    nc.scalar.activation(
                        out=kvT[:, sl], in_=psum_kv[nt][:],
                        func=Identity, bias=bkv_sb[:, 0:1],
                    )
                else:
                    nc.vector.tensor_add(
                        out=kvT[:, sl],
                        in0=psum_kv[nt][:],
                        in1=bkv_sb.to_broadcast((128, 512)),
                    )

            # Back-transpose kv^T to token-major for the S contraction —
            # on the DMA crossbar (PE via identity as fallback).
            kv_nat = dpool.tile([128, TI, 128], f16)
            for ti in range(TI):
                tsl = slice(ti * 128, (ti + 1) * 128)
                if pe_transpose:
                    pkv_t = ppool.tile([128, 128], f16, tag="A", name="pkvt")
                    nc.tensor.transpose(pkv_t[:], kvT[:, tsl], id_sb[:])
                    nc.vector.tensor_copy(out=kv_nat[:, ti, :], in_=pkv_t[:])
                else:
                    nc.sync.dma_start_transpose(
                        out=kv_nat[:, ti, :], in_=kvT[:, tsl]
                    )

            # Partial S = k^T v over this core's 2048 tokens.
            psum_s = ppool.tile([H, H], f32, tag="B", name="ps")
            for ti in range(TI):
                nc.tensor.matmul(
                    psum_s[:],
                    kv_nat[:, ti, 0:H],
                    kv_nat[:, ti, H : 2 * H],
                    start=(ti == 0),
                    stop=(ti == TI - 1),
                )
            s_sb = wpool.tile([H, H], f16, tag="s")
            nc.vector.tensor_copy(out=s_sb[:], in_=psum_s[:])
            dma_to_cc = nc.sync.dma_start(out=cc_in.ap(), in_=s_sb[:])

            if not no_collective:
                # AllGather of the fp16 partial (latency-bound: smaller is
                # faster).  The pair sum minus the local partial gives the
                # partner's S without needing the core's rank.  (Issuing
                # four single-pair collectives deadlocks NRT — cores outside
                # a replica group do not no-op the instruction.)
                cc = nc.gpsimd.collective_compute(
                    "AllGather",
                    mybir.AluOpType.bypass,
                    replica_groups=RG,
                    ins=[cc_in.ap()],
                    outs=[cc_out.ap()],
                )
                add_dep_helper(
                    cc.ins, dma_to_cc.ins, reason="collective waits for S DMA"
                )
                sg_sb = wpool.tile([H, 2, H], f16, tag="sg")
                dma_from_cc = nc.sync.dma_start(
                    out=sg_sb[:], in_=cc_out.ap().rearrange("r p h -> p r h")
                )
                add_dep_helper(
                    dma_from_cc.ins, cc.ins, reason="S readback waits for collective"
                )

            # q^T = (scale*Wq)^T x^T + scale*bq, overlapping the collective.
            for k, ci in enumerate(CI_ORDER):
                for nt in range(NT):
                    sl = slice(nt * 512, (nt + 1) * 512)
                    nc.tensor.matmul(
                        psum_q[nt][:], wq_sb[:, ci, :], xt_sb[ci][:, sl],
                        start=(k == 0), stop=(k == CI - 1),
                    )
            for nt in range(NT):
                sl = slice(nt * 512, (nt + 1) * 512)
                if nt % 2 == 0:
                    nc.scalar.activation(
                        out=qT[:, sl], in_=psum_q[nt][:],
                        func=Identity, bias=bq_sb[:, 0:1],
                    )
                else:
                    nc.vector.tensor_add(
                        out=qT[:, sl],
                        in0=psum_q[nt][:],
                        in1=bq_sb.to_broadcast((H, 512)),
                    )

            # out = (scale*q) @ S_full; 16 ti-outputs pack into 2 psum
            # banks so the epilogue is 2 wide copies + 2 DMAs.
            sf16 = wpool.tile([H, H], f16, tag="sf16")
            if no_collective:
                nc.vector.tensor_copy(out=sf16[:], in_=s_sb[:])
            else:
                # Pair-sum fused in one DVE op.
                nc.vector.tensor_add(
                    out=sf16[:], in0=sg_sb[:, 0, :], in1=sg_sb[:, 1, :]
                )

            po_big = [
                ppool.tile([128, 8 * H], f32, tag="A", name=f"pob{g}")
                for g in range(2)
            ]
            out_sb = dpool.tile([128, 2, 8 * H], f16)
            for ti in range(TI):
                tsl = slice(ti * 128, (ti + 1) * 128)
                osl = slice((ti % 8) * H, (ti % 8 + 1) * H)
                nc.tensor.matmul(
                    po_big[ti // 8][:, osl], qT[:, tsl], sf16[:],
                    start=True, stop=True,
                )
            nc.vector.tensor_copy(out=out_sb[:, 0, :], in_=po_big[0][:])
            nc.scalar.dma_start(out=out[:, 0, :], in_=out_sb[:, 0, :])
            nc.scalar.activation(
                out=out_sb[:, 1, :], in_=po_big[1][:],
                func=mybir.ActivationFunctionType.Copy, bias=0.0,
            )
            nc.sync.dma_start(out=out[:, 1, :], in_=out_sb[:, 1, :])

    if walrus_patches:
        _dedup_ldweights(nc)
        _split_multi_waits(nc)
    return nc


def _make_runner(**build_kwargs):
    """Build the Bass module once and wrap it in a cached, jitted PJRT
    executable.  Output buffers are created on-device inside the jit (no
    host->device zero upload)."""
    import jax
    import jax.numpy as jnp
    from jax.experimental.shard_map import shard_map
    from jax.sharding import Mesh, NamedSharding, PartitionSpec

    import concourse.mybir as mybir
    from concourse import bass2jax

    nc = _build_nc(**build_kwargs)
    bass2jax.install_neuronx_cc_hook()

    partition_name = nc.partition_id_tensor.name if nc.partition_id_tensor else None
    in_names, out_names, out_avals, zero_shapes = [], [], [], []
    for alloc in nc.m.functions[0].allocations:
        if not isinstance(alloc, mybir.MemoryLocationSet):
            continue
        name = alloc.memorylocations[0].name
        if alloc.kind == "ExternalInput":
            if name != partition_name:
                in_names.append(name)
        elif alloc.kind == "ExternalOutput":
            out_names.append(name)
            shape = tuple(alloc.tensor_shape)
            dtype = mybir.dt.np(alloc.dtype)
            out_avals.append(jax.core.ShapedArray(shape, dtype))
            zero_shapes.append((shape, dtype))
    n_params = len(in_names)
    in_names_all = list(in_names) + list(out_names)
    if partition_name:
        in_names_all.append(partition_name)

    def _body(*args):
        operands = list(args)
        if partition_name:
            operands.append(bass2jax.partition_id_tensor())
        outs = bass2jax._bass_exec_p.bind(
            *operands,
            out_avals=tuple(out_avals),
            in_names=tuple(in_names_all),
            out_names=tuple(out_names),
            lowering_input_output_aliases=(),
            sim_require_finite=True,
            sim_require_nnan=True,
            nc=nc,
        )
        return tuple(outs)

    devices = jax.devices()[:N_CORES]
    assert len(devices) == N_CORES
    mesh = Mesh(np.asarray(devices), ("core",))
    sharding = NamedSharding(mesh, PartitionSpec("core"))
    n_outs = len(out_names)
    sharded = jax.jit(
        shard_map(
            _body,
            mesh=mesh,
            in_specs=(PartitionSpec("core"),) * (n_params + n_outs),
            out_specs=(PartitionSpec("core"),) * n_outs,
            check_rep=False,
        ),
        donate_argnums=tuple(range(n_params, n_params + n_outs)),
        keep_unused=True,
    )
    return {
        "nc": nc,
        "sharded": sharded,
        "sharding": sharding,
        "in_names": in_names,
        "out_names": out_names,
        "out_avals": out_avals,
        "zero_shapes": zero_shapes,
        "out_bufs": None,  # device-resident recycled output buffers
    }


def _get_runner(**build_kwargs):
    key = ("runner", tuple(sorted(build_kwargs.items())))
    if key not in _CACHE:
        _CACHE[key] = _make_runner(**build_kwargs)
    return _CACHE[key]


def _prep_consts(runner, Wq, bq, Wk, bk, Wv, bv):
    """Marshal + device_put the weight/constant tensors, cached by content
    hash so repeat calls with unchanged weights skip the upload."""
    import jax

    h = hashlib.blake2b(digest_size=16)
    for a in (Wq, bq, Wk, bk, Wv, bv):
        h.update(np.ascontiguousarray(a).tobytes())
    key = ("consts", h.hexdigest())
    if key in _CACHE:
        return _CACHE[key]

    Wq = np.asarray(Wq, np.float32)
    Wk = np.asarray(Wk, np.float32)
    Wv = np.asarray(Wv, np.float32)
    bq_ = np.asarray(bq, np.float32)
    bk_ = np.asarray(bk, np.float32)
    bv_ = np.asarray(bv, np.float32)

    wkv = np.concatenate([Wk, Wv], axis=1)  # [768, 128]
    wkv_r = wkv.reshape(CI, 128, 128).transpose(1, 0, 2).astype(np.float16)
    wq_r = (Wq * SCALE).reshape(CI, 128, H).transpose(1, 0, 2).astype(np.float16)
    bkv = np.concatenate([bk_, bv_])[:, None].astype(np.float32)
    bq_r = (bq_ * SCALE)[:, None].astype(np.float32)
    id16 = np.eye(128, dtype=np.float16)

    per_core = {
        "wkv": np.ascontiguousarray(wkv_r),
        "wq": np.ascontiguousarray(wq_r),
        "bkv": bkv,
        "bq": bq_r,
        "ident": id16,
    }
    devs = {
        nm: jax.device_put(
            np.concatenate([per_core[nm]] * N_CORES, axis=0), runner["sharding"]
        )
        for nm in per_core
    }
    _CACHE[key] = devs
    return devs


def _executor():
    from concurrent.futures import ThreadPoolExecutor

    if "pool" not in _CACHE:
        _CACHE["pool"] = ThreadPoolExecutor(N_CORES)
    return _CACHE["pool"]


def _put_x(x, runner):
    """Marshal + upload x per core, pipelined across threads: each core's
    fp32->fp16 transposing cast overlaps the other cores' tunnel transfers.
    The transpose rides inside the cast for free."""
    import jax

    x = np.asarray(x)
    devices = jax.devices()[:N_CORES]

    def one(c):
        b, h = divmod(c, 2)
        xc = x[b, h * TPC : (h + 1) * TPC, :]
        xt = xc.reshape(TPC, CI, 128).transpose(2, 1, 0).astype(np.float16)
        return jax.device_put(xt, devices[c])

    arrs = list(_executor().map(one, range(N_CORES)))
    return jax.make_array_from_single_device_arrays(
        (N_CORES * 128, CI, TPC), runner["sharding"], arrs
    )


def _assemble(out_np):
    """[N_CORES*128, 2, 8H] fp16 -> [B, T, H] fp32."""
    oc = out_np.reshape(B, 2, 128, 2, 8, H)
    # token within half = g*1024 + k*128 + p  (g psum group, k tile-in-group)
    full = oc.transpose(0, 1, 3, 4, 2, 5).astype(np.float32)
    return full.reshape(B, T, H)


def kernel(**inputs):
    import jax

    runner = _get_runner()
    consts = _prep_consts(
        runner,
        inputs["Wq"], inputs["bq"],
        inputs["Wk"], inputs["bk"],
        inputs["Wv"], inputs["bv"],
    )
    x_dev = _put_x(inputs["x"], runner)
    args = [x_dev if nm == "xt" else consts[nm] for nm in runner["in_names"]]
    # The kernel fully overwrites its output tensors, so their incoming
    # contents are irrelevant: recycle the previous call's device-resident
    # outputs as this call's donated buffers (first call uploads zeros once).
    out_bufs = runner["out_bufs"]
    if out_bufs is None:
        out_bufs = [
            jax.device_put(
                np.zeros((N_CORES * s[0], *s[1:]), d), runner["sharding"]
            )
            for s, d in runner["zero_shapes"]
        ]
    outs = runner["sharded"](*args, *out_bufs)
    shards = sorted(outs[0].addressable_shards, key=lambda s: s.index[0].start)
    parts = list(_executor().map(lambda s: np.asarray(s.data), shards))
    out_np = np.concatenate(parts, axis=0)
    runner["out_bufs"] = list(outs)
    return _assemble(out_np)


# revision 41
# speedup vs baseline: 1.0407x; 1.0407x over previous
"""Trainium2 Bass kernel for nn_AttentionHead (pre-softmax scores variant).

The module returns (q @ k^T * scale) @ v with NO softmax, so the product is
associative:  out = (scale*q) @ (k^T @ v)  with k^T @ v a tiny [64, 64]
matrix.  This removes the [T, T] score matrix entirely: the kernel streams
x once, computes k/v/q projections, a [64, 64] partial S = k^T v, a pairwise
AllGather+add across the two cores holding each batch, and one final
tall-skinny matmul.

Sharding: core c <- (batch b = c//2, sequence half h = c%2), 2048 tokens per
core.  Partial S matrices are exchanged within core pairs
[[0,1],[2,3],[4,5],[6,7]].

The wall-clock cost of a call is dominated by the axon tunnel (~60 MB/s
aggregate), so the host<->device byte count is the primary optimization
axis: x ships as fp16 (24 MB instead of 48), already transposed host-side
(the transpose fuses into the fp32->fp16 cast for free), weights/constants
are content-hashed and kept device-resident across calls, output buffers
are created on-device inside the jit, and the output returns as fp16
(2 MB).  fp16 keeps the end-to-end relative error ~5e-4.
"""

import hashlib
import sys

sys.path.insert(0, "/opt/trn_rl_repo")

import numpy as np

B, T, C, H = 4, 4096, 768, 64
N_CORES = 8
TPC = T // 2  # tokens per core (half a batch's sequence)
CI = C // 128  # 6 contraction chunks
NT = TPC // 512  # 4 moving-dim slices for projections
TI = TPC // 128  # 16 token tiles
SCALE = float(C) ** -0.5

_CACHE = {}


def _patch_tile_drain():
    """This walrus build rejects >1 sync wait on TPB_CTRL instructions
    (Drain/NoOp) and the butterfly barrier rides eq-waits on drains.
    Replace the TileContext exit sequence with single-wait nops + plain
    drain + sem-only barriers."""
    import bass_rust as _bass_rust
    import concourse.tile as tile
    from concourse.vector_clock import ScopedClock

    def _drain_and_barrier(self, tick_clock, wait_clock):
        nc = self.nc
        probe = nc.sync.nop(nofuse=True)
        wait_clock.add_sem_waits(
            probe.ins, ScopedClock({None: tick_clock.global_clock})
        )
        waits = list(probe.ins.sync_info.on_wait) if probe.ins.sync_info else []
        updates = list(probe.ins.sync_info.on_update) if probe.ins.sync_info else []
        probe.ins.sync_info = _bass_rust.SyncInfo(
            on_wait=waits[:1], on_update=updates
        )
        for i in range(1, len(waits)):
            extra = nc.sync.nop(nofuse=True)
            extra.ins.sync_info = _bass_rust.SyncInfo(
                on_wait=waits[i : i + 1], on_update=[]
            )
        nc.sync.drain()
        nc.all_engine_barrier(sem_only=True)
        popped = nc._tile_sem_poison_stack.pop()
        assert popped is self._sem_poison
        # Skip the exit-time semaphore clear + second barrier (~2us of
        # counted exec span per core): the NEFF prologue re-initializes
        # semaphore state on every execution — repeat-call correctness
        # verified with this exact exit sequence (rel err unchanged over
        # REPS=5 + multiple profiled runs).

    tile.TileContext._drain_and_barrier = _drain_and_barrier


def _split_multi_waits(nc):
    """This walrus build allows only ONE sync-wait command per regular
    instruction.  Move extra waits onto dedicated same-engine NOPs placed
    immediately before the instruction (an engine blocks on its own stream,
    so this is semantically identical)."""
    import bass_rust
    import concourse.mybir as mybir

    cnt = 0
    for fn in nc.m.functions:
        for bb in fn.blocks:
            out = []
            for ins in bb.instructions:
                si = ins.sync_info
                if si is not None and si.on_wait and len(si.on_wait) > 1:
                    waits = list(si.on_wait)
                    for w in waits[:-1]:
                        nop = mybir.InstNoOp(name=f"I-waitsplit-{cnt}")
                        cnt += 1
                        nop.engine = ins.engine
                        nop.bass_nofuse = True
                        nop.sync_info = bass_rust.SyncInfo(
                            on_wait=[w], on_update=[]
                        )
                        out.append(nop)
                    ins.sync_info = bass_rust.SyncInfo(
                        on_wait=[waits[-1]], on_update=list(si.on_update or [])
                    )
                out.append(ins)
            bb.instructions = out
    return cnt


def _dedup_ldweights(nc):
    """Tile lowers every non-fp32 matmul into an LDWEIGHTS+MATMUL pair.
    When consecutive PE matmuls share the identical stationary operand the
    reload is redundant (the array already holds it) — delete those
    LDWEIGHTS, reattaching any sync waits to the next instruction."""
    import bass_rust

    def wkey(pap):
        return (str(pap.ap), pap.offset, str(pap.memref))

    removed = 0
    for fn in nc.m.functions:
        for bb in fn.blocks:
            out = []
            last_w = None
            pending_waits = []
            for ins in bb.instructions:
                nm = type(ins).__name__
                if nm == "InstLdweights":
                    k = wkey(ins.ins[0])
                    if last_w == k:
                        if ins.sync_info and ins.sync_info.on_wait:
                            pending_waits.extend(ins.sync_info.on_wait)
                        if ins.sync_info and ins.sync_info.on_update:
                            out.append(ins)
                            last_w = k
                            continue
                        removed += 1
                        continue
                    last_w = k
                elif nm == "InstMatmult":
                    if ins.is_transpose:
                        last_w = None  # transpose streams data through the array
                    else:
                        last_w = wkey(ins.ins[1])
                elif nm in ("InstCompareAndBranch", "InstUnconditionalBranch",
                            "InstCall", "InstDrain"):
                    last_w = None
                if pending_waits and ins.engine is not None:
                    w = list(pending_waits)
                    if ins.sync_info:
                        w = list(ins.sync_info.on_wait) + w
                        upd = list(ins.sync_info.on_update)
                    else:
                        upd = []
                    ins.sync_info = bass_rust.SyncInfo(on_wait=w, on_update=upd)
                    pending_waits = []
                out.append(ins)
            bb.instructions = out
    return removed


def _build_nc(no_collective=False, walrus_patches=True, pe_transpose=True):
    import concourse.bass as bass
    import concourse.mybir as mybir
    import concourse.tile as tile
    from bass_rust import add_dep_helper

    _patch_tile_drain()

    f32 = mybir.dt.float32
    f16 = mybir.dt.float16
    Identity = mybir.ActivationFunctionType.Identity

    nc = bass.Bass("TRN2", target_bir_lowering=False, debug=False,
                   num_devices=N_CORES)

    xt = nc.dram_tensor("xt", [128, CI, TPC], f16, kind="ExternalInput").ap()
    wkv = nc.dram_tensor("wkv", [128, CI, 128], f16, kind="ExternalInput").ap()
    wq = nc.dram_tensor("wq", [128, CI, H], f16, kind="ExternalInput").ap()
    bkv = nc.dram_tensor("bkv", [128, 1], f32, kind="ExternalInput").ap()
    bqp = nc.dram_tensor("bq", [H, 1], f32, kind="ExternalInput").ap()
    ident = nc.dram_tensor("ident", [128, 128], f16, kind="ExternalInput").ap()
    out = nc.dram_tensor("out", [128, 2, 8 * H], f16, kind="ExternalOutput").ap()
    cc_in = nc.dram_tensor("cc_in", [H, H], f16)
    cc_out = nc.dram_tensor("cc_out", [2, H, H], f16)
    cc_warm_in = nc.dram_tensor("cc_warm_in", [1, 1], f32)
    cc_warm_out = nc.dram_tensor("cc_warm_out", [2, 1], f32)
    RG = [[0, 1], [2, 3], [4, 5], [6, 7]]

    with tile.TileContext(nc) as tc:
        with (
            tc.tile_pool(name="const", bufs=1) as cpool,
            tc.tile_pool(name="data", bufs=1) as dpool,
            tc.tile_pool(name="work", bufs=2) as wpool,
            tc.tile_pool(name="psum", bufs=4, space="PSUM") as ppool,
        ):
            # DMA order tuned for the kv critical path: wkv heads the SP
            # queue so the first matmul is gated only on it + chunk 0;
            # chunk 0 lands in matmul-sized quarters so the PE starts
            # after 128 KB.  Even chunks ride SP, odd chunks + the
            # late-needed consts ride Act.  Each chunk gets its own tile
            # so matmuls gate on per-chunk DMA completion.
            # Warm-up collective: absorbs the CC-stream startup latency and
            # barrier-aligns the cores while x is still streaming, so the
            # real S exchange later pays only the transfer.  Contents are
            # irrelevant (uninitialized DRAM round-trip, never read).
            if not no_collective:
                nc.gpsimd.collective_compute(
                    "AllGather",
                    mybir.AluOpType.bypass,
                    replica_groups=RG,
                    ins=[cc_warm_in.ap()],
                    outs=[cc_warm_out.ap()],
                )

            wkv_sb = cpool.tile([128, CI, 128], f16)
            nc.sync.dma_start(out=wkv_sb[:], in_=wkv)
            bkv_sb = cpool.tile([128, 1], f32)
            nc.scalar.dma_start(out=bkv_sb[:], in_=bkv)
            bq_sb = cpool.tile([H, 1], f32)
            nc.scalar.dma_start(out=bq_sb[:], in_=bqp)
            wq_sb = cpool.tile([128, CI, H], f16)
            nc.scalar.dma_start(out=wq_sb[:], in_=wq)

            # Preload the Act engine's PWP table off the critical path (it
            # otherwise loads lazily right before the first bias ACTIVATE).
            act_dummy = cpool.tile([1, 1], f32)
            nc.scalar.activation(
                out=act_dummy[:], in_=bkv_sb[0:1, 0:1],
                func=Identity, bias=bkv_sb[0:1, 0:1],
            )

            # Queue balance: chunks 0-3 are split in half across BOTH HWDGE
            # queues (halving each chunk's arrival latency); the fast SWDGE
            # burst queue carries c4 + c5 whole.  The kv loop consumes
            # chunks in expected-arrival order.
            xt_sb = [dpool.tile([128, TPC], f16, name=f"xt{ci}") for ci in range(CI)]
            nc.gpsimd.dma_start(out=xt_sb[4][:], in_=xt[:, 4, :])
            nc.gpsimd.dma_start(out=xt_sb[5][:], in_=xt[:, 5, :])
            id_sb = cpool.tile([128, 128], f16)
            if pe_transpose:
                nc.scalar.dma_start(out=id_sb[:], in_=ident)
            for ci in range(4):
                nc.sync.dma_start(
                    out=xt_sb[ci][:, 0:1024], in_=xt[:, ci, 0:1024]
                )
                nc.scalar.dma_start(
                    out=xt_sb[ci][:, 1024:2048], in_=xt[:, ci, 1024:2048]
                )
            CI_ORDER = [0, 4, 1, 5, 2, 3]

            # kv^T = (Wk | Wv)^T x^T + bias (rows 0..63 k^T, 64..127 v^T).
            # kv first so the S collective launches before the q work.
            kvT = dpool.tile([128, TPC], f16)
            qT = dpool.tile([H, TPC], f16)
            psum_kv = [
                ppool.tile([128, 512], f32, tag="A", name=f"pkv{nt}")
                for nt in range(NT)
            ]
            psum_q = [
                ppool.tile([H, 512], f32, tag="B", name=f"pq{nt}")
                for nt in range(NT)
            ]
            for k, ci in enumerate(CI_ORDER):
                for nt in range(NT):
                    sl = slice(nt * 512, (nt + 1) * 512)
                    nc.tensor.matmul(
                        psum_kv[nt][:], wkv_sb[:, ci, :], xt_sb[ci][:, sl],
                        start=(k == 0), stop=(k == CI - 1),
                    )
            for nt in range(NT):
                sl = slice(nt * 512, (nt + 1) * 512)
                if nt % 2 == 0:
                    nc.scalar.activation(
                        out=kvT[:, sl], in_=psum_kv[nt][:],
                        func=Identity, bias=bkv_sb[:, 0:1],
                    )
                else:
                    nc.vector.tensor_add(
                        out=kvT[:, sl],
                        in0=psum_kv[nt][:],
                        in1=bkv_sb.to_broadcast((128, 512)),
                    )

            # Back-transpose kv^T to token-major for the S contraction —
            # on the DMA crossbar (PE via identity as fallback).
            kv_nat = dpool.tile([128, TI, 128], f16)
            for ti in range(TI):
                tsl = slice(ti * 128, (ti + 1) * 128)
                if pe_transpose:
                    pkv_t = ppool.tile([128, 128], f16, tag="A", name="pkvt")
                    nc.tensor.transpose(pkv_t[:], kvT[:, tsl], id_sb[:])
                    nc.vector.tensor_copy(out=kv_nat[:, ti, :], in_=pkv_t[:])
                else:
                    nc.sync.dma_start_transpose(
                        out=kv_nat[:, ti, :], in_=kvT[:, tsl]
                    )

            # Partial S = k^T v over this core's 2048 tokens.
            psum_s = ppool.tile([H, H], f32, tag="B", name="ps")
            for ti in range(TI):
                nc.tensor.matmul(
                    psum_s[:],
                    kv_nat[:, ti, 0:H],
                    kv_nat[:, ti, H : 2 * H],
                    start=(ti == 0),
                    stop=(ti == TI - 1),
                )
            s_sb = wpool.tile([H, H], f16, tag="s")
            nc.vector.tensor_copy(out=s_sb[:], in_=psum_s[:])
            dma_to_cc = nc.sync.dma_start(out=cc_in.ap(), in_=s_sb[:])

            if not no_collective:
                # AllGather of the fp16 partial (latency-bound: smaller is
                # faster).  The pair sum minus the local partial gives the
                # partner's S without needing the core's rank.  (Issuing
                # four single-pair collectives deadlocks NRT — cores outside
                # a replica group do not no-op the instruction.)
                cc = nc.gpsimd.collective_compute(
                    "AllGather",
                    mybir.AluOpType.bypass,
                    replica_groups=RG,
                    ins=[cc_in.ap()],
                    outs=[cc_out.ap()],
                )
                add_dep_helper(
                    cc.ins, dma_to_cc.ins, reason="collective waits for S DMA"
                )
                sg_sb = wpool.tile([H, 2, H], f16, tag="sg")
                dma_from_cc = nc.sync.dma_start(
                    out=sg_sb[:], in_=cc_out.ap().rearrange("r p h -> p r h")
                )
                add_dep_helper(
                    dma_from_cc.ins, cc.ins, reason="S readback waits for collective"
                )

            # q^T = (scale*Wq)^T x^T + scale*bq, overlapping the collective.
            for k, ci in enumerate(CI_ORDER):
                for nt in range(NT):
                    sl = slice(nt * 512, (nt + 1) * 512)
                    nc.tensor.matmul(
                        psum_q[nt][:], wq_sb[:, ci, :], xt_sb[ci][:, sl],
                        start=(k == 0), stop=(k == CI - 1),
                    )
            for nt in range(NT):
                sl = slice(nt * 512, (nt + 1) * 512)
                if nt % 2 == 0:
                    nc.scalar.activation(
                        out=qT[:, sl], in_=psum_q[nt][:],
                        func=Identity, bias=bq_sb[:, 0:1],
                    )
                else:
                    nc.vector.tensor_add(
                        out=qT[:, sl],
                        in0=psum_q[nt][:],
                        in1=bq_sb.to_broadcast((H, 512)),
                    )

            # out = (scale*q) @ S_full; 16 ti-outputs pack into 2 psum
            # banks so the epilogue is 2 wide copies + 2 DMAs.
            sf16 = wpool.tile([H, H], f16, tag="sf16")
            if no_collective:
                nc.vector.tensor_copy(out=sf16[:], in_=s_sb[:])
            else:
                # Pair-sum fused in one DVE op.
                nc.vector.tensor_add(
                    out=sf16[:], in0=sg_sb[:, 0, :], in1=sg_sb[:, 1, :]
                )

            po_big = [
                ppool.tile([128, 8 * H], f32, tag="A", name=f"pob{g}")
                for g in range(2)
            ]
            out_sb = dpool.tile([128, 2, 8 * H], f16)
            for ti in range(TI):
                tsl = slice(ti * 128, (ti + 1) * 128)
                osl = slice((ti % 8) * H, (ti % 8 + 1) * H)
                nc.tensor.matmul(
                    po_big[ti // 8][:, osl], qT[:, tsl], sf16[:],
                    start=True, stop=True,
                )
            nc.vector.tensor_copy(out=out_sb[:, 0, :], in_=po_big[0][:])
            nc.scalar.dma_start(out=out[:, 0, :], in_=out_sb[:, 0, :])
            nc.scalar.activation(
                out=out_sb[:, 1, :], in_=po_big[1][:],
                func=mybir.ActivationFunctionType.Copy, bias=0.0,
            )
            nc.sync.dma_start(out=out[:, 1, :], in_=out_sb[:, 1, :])

    if walrus_patches:
        _dedup_ldweights(nc)
        _split_multi_waits(nc)
    return nc


def _make_runner(**build_kwargs):
    """Build the Bass module once and wrap it in a cached, jitted PJRT
    executable.  Output buffers are created on-device inside the jit (no
    host->device zero upload)."""
    import jax
    import jax.numpy as jnp
    from jax.experimental.shard_map import shard_map
    from jax.sharding import Mesh, NamedSharding, PartitionSpec

    import concourse.mybir as mybir
    from concourse import bass2jax

    nc = _build_nc(**build_kwargs)
    bass2jax.install_neuronx_cc_hook()

    partition_name = nc.partition_id_tensor.name if nc.partition_id_tensor else None
    in_names, out_names, out_avals, zero_shapes = [], [], [], []
    for alloc in nc.m.functions[0].allocations:
        if not isinstance(alloc, mybir.MemoryLocationSet):
            continue
        name = alloc.memorylocations[0].name
        if alloc.kind == "ExternalInput":
            if name != partition_name:
                in_names.append(name)
        elif alloc.kind == "ExternalOutput":
            out_names.append(name)
            shape = tuple(alloc.tensor_shape)
            dtype = mybir.dt.np(alloc.dtype)
            out_avals.append(jax.core.ShapedArray(shape, dtype))
            zero_shapes.append((shape, dtype))
    n_params = len(in_names)
    in_names_all = list(in_names) + list(out_names)
    if partition_name:
        in_names_all.append(partition_name)

    def _body(*args):
        operands = list(args)
        if partition_name:
            operands.append(bass2jax.partition_id_tensor())
        outs = bass2jax._bass_exec_p.bind(
            *operands,
            out_avals=tuple(out_avals),
            in_names=tuple(in_names_all),
            out_names=tuple(out_names),
            lowering_input_output_aliases=(),
            sim_require_finite=True,
            sim_require_nnan=True,
            nc=nc,
        )
        return tuple(outs)

    devices = jax.devices()[:N_CORES]
    assert len(devices) == N_CORES
    mesh = Mesh(np.asarray(devices), ("core",))
    sharding = NamedSharding(mesh, PartitionSpec("core"))
    n_outs = len(out_names)
    sharded = jax.jit(
        shard_map(
            _body,
            mesh=mesh,
            in_specs=(PartitionSpec("core"),) * (n_params + n_outs),
            out_specs=(PartitionSpec("core"),) * n_outs,
            check_rep=False,
        ),
        donate_argnums=tuple(range(n_params, n_params + n_outs)),
        keep_unused=True,
    )
    return {
        "nc": nc,
        "sharded": sharded,
        "sharding": sharding,
        "in_names": in_names,
        "out_names": out_names,
        "out_avals": out_avals,
        "zero_shapes": zero_shapes,
        "out_bufs": None,  # device-resident recycled output buffers
    }


def _get_runner(**build_kwargs):
    key = ("runner", tuple(sorted(build_kwargs.items())))
    if key not in _CACHE:
        _CACHE[key] = _make_runner(**build_kwargs)
    return _CACHE[key]


def _prep_consts(runner, Wq, bq, Wk, bk, Wv, bv):
    """Marshal + device_put the weight/constant tensors, cached by content
    hash so repeat calls with unchanged weights skip the upload."""
    import jax

    h = hashlib.blake2b(digest_size=16)
    for a in (Wq, bq, Wk, bk, Wv, bv):
        h.update(np.ascontiguousarray(a).tobytes())
    key = ("consts", h.hexdigest())
    if key in _CACHE:
        return _CACHE[key]

    Wq = np.asarray(Wq, np.float32)
    Wk = np.asarray(Wk, np.float32)
    Wv = np.asarray(Wv, np.float32)
    bq_ = np.asarray(bq, np.float32)
    bk_ = np.asarray(bk, np.float32)
    bv_ = np.asarray(bv, np.float32)

    wkv = np.concatenate([Wk, Wv], axis=1)  # [768, 128]
    wkv_r = wkv.reshape(CI, 128, 128).transpose(1, 0, 2).astype(np.float16)
    wq_r = (Wq * SCALE).reshape(CI, 128, H).transpose(1, 0, 2).astype(np.float16)
    bkv = np.concatenate([bk_, bv_])[:, None].astype(np.float32)
    bq_r = (bq_ * SCALE)[:, None].astype(np.float32)
    id16 = np.eye(128, dtype=np.float16)

    per_core = {
        "wkv": np.ascontiguousarray(wkv_r),
        "wq": np.ascontiguousarray(wq_r),
        "bkv": bkv,
        "bq": bq_r,
        "ident": id16,
    }
    devs = {
        nm: jax.device_put(
            np.concatenate([per_core[nm]] * N_CORES, axis=0), runner["sharding"]
        )
        for nm in per_core
    }
    _CACHE[key] = devs
    return devs


def _executor():
    from concurrent.futures import ThreadPoolExecutor

    if "pool" not in _CACHE:
        _CACHE["pool"] = ThreadPoolExecutor(N_CORES)
    return _CACHE["pool"]


def _put_x(x, runner):
    """Marshal + upload x per core, pipelined across threads: each core's
    fp32->fp16 transposing cast overlaps the other cores' tunnel transfers.
    The transpose rides inside the cast for free."""
    import jax

    x = np.asarray(x)
    devices = jax.devices()[:N_CORES]

    def one(c):
        b, h = divmod(c, 2)
        xc = x[b, h * TPC : (h + 1) * TPC, :]
        xt = xc.reshape(TPC, CI, 128).transpose(2, 1, 0).astype(np.float16)
        return jax.device_put(xt, devices[c])

    arrs = list(_executor().map(one, range(N_CORES)))
    return jax.make_array_from_single_device_arrays(
        (N_CORES * 128, CI, TPC), runner["sharding"], arrs
    )


def _assemble(out_np):
    """[N_CORES*128, 2, 8H] fp16 -> [B, T, H] fp32."""
    oc = out_np.reshape(B, 2, 128, 2, 8, H)
    # token within half = g*1024 + k*128 + p  (g psum group, k tile-in-group)
    full = oc.transpose(0, 1, 3, 4, 2, 5).astype(np.float32)
    return full.reshape(B, T, H)


def kernel(**inputs):
    import jax

    runner = _get_runner()
    consts = _prep_consts(
        runner,
        inputs["Wq"], inputs["bq"],
        inputs["Wk"], inputs["bk"],
        inputs["Wv"], inputs["bv"],
    )
    x_dev = _put_x(inputs["x"], runner)
    args = [x_dev if nm == "xt" else consts[nm] for nm in runner["in_names"]]
    # The kernel fully overwrites its output tensors, so their incoming
    # contents are irrelevant: recycle the previous call's device-resident
    # outputs as this call's donated buffers (first call uploads zeros once).
    out_bufs = runner["out_bufs"]
    if out_bufs is None:
        out_bufs = [
            jax.device_put(
                np.zeros((N_CORES * s[0], *s[1:]), d), runner["sharding"]
            )
            for s, d in runner["zero_shapes"]
        ]
    outs = runner["sharded"](*args, *out_bufs)
    shards = sorted(outs[0].addressable_shards, key=lambda s: s.index[0].start)
    parts = list(_executor().map(lambda s: np.asarray(s.data), shards))
    out_np = np.concatenate(parts, axis=0)
    runner["out_bufs"] = list(outs)
    return _assemble(out_np)
